# revision 20
# baseline (speedup 1.0000x reference)
"""Trainium2 Bass kernel for nn_Architecture_79353815760942 (decay-attention
dense transformer, B=4 S=1024 D=1024 H=16, mean+cov twin pipelines).

Sharding: 8 cores = 2 tensor-types (mean, cov) x 4 batches. The mean and cov
pipelines are fully independent, so each core runs one (type, batch) slice
end-to-end with zero inter-core communication.

Per-core pipeline (all layouts chosen so no on-chip transposes of activations
are needed except the attention-probability tiles, which go through the PE
transpose unit):
  phase 1: q/k/v projections from host-pre-transposed bf16 inputs
           qmT,kmT in [d_head, seq] layout; v in [seq, d_head] layout
  phase 2: per head: scores -> exp -> segmented-scan cumsum -> decay factor
           (via ln/exp to stay in one ACT table set) -> second softmax ->
           PE-transpose -> P^T V accumulation (omT layout)
  phase 3: output projection from omT, f32 DMA out.

Engine budget (per core, measured-calibrated):
  PE: projections + 2x scores + PV + PE-transposes, emitted interleaved
      across head pairs so the tensor engine never idles (p-state ramp).
  ACT: the three exp passes.
  DVE: scan, s2-mult (in-psum), shift-sqrt, e2 normalize, psum drains.
  Pool: u-step STT, diagonal mask adds, projection psum drains.
"""
import os
import numpy as np
import ml_dtypes

from concourse import bass, bacc, tile, mybir
from concourse.bass_utils import run_bass_kernel_spmd

BF16 = ml_dtypes.bfloat16
F32 = np.float32

B, S, D, H = 4, 1024, 1024, 16
DK = D // H          # 64
NT = S // 128        # 8 query tiles per (b,h)
OFFS = [128 * qb * (qb + 1) // 2 for qb in range(NT)]   # mega col offset per q tile
MEGA = OFFS[-1] + 128 * NT  # 4608
NEG = -1e30
LN1_16 = float(np.log(1.0 / 16.0))  # exp1 bias: keeps e1/suffix sums in fp16 range

_cache = {}


def _ceil_div(a, b):
    return (a + b - 1) // b


def build_bass(do_attn=True, do_out=True, n_hp=8, dbg_dump=(),
               scan_split=None, scan_split_engine="vector",
               pool_shift=False, pool_norm=False, pool_u0=True):
    """Build the SPMD single-core program (same graph on all 8 cores).

    Emission order software-pipelines the head pairs: pass A of pair hp+1 is
    emitted before pass B of pair hp, so the tensor engine always has matmul
    work while pair hp's scan/decay chain runs on DVE/Pool/ACT.

    scan_split: optional mega-col tile boundary (an OFFS value); the scan
    for cols [split, MEGA) runs on scan_split_engine instead of DVE. The
    per-row diagonal resets make tile segments independent, so any tile
    boundary is a safe split point.
    """
    fp32 = mybir.dt.float32
    bf16 = mybir.dt.bfloat16
    fp16 = mybir.dt.float16

    nc = bacc.Bacc("TRN2", target_bir_lowering=False, debug=False, num_devices=8)

    def din(name, shape, dt):
        return nc.dram_tensor(name, list(shape), dt, kind="ExternalInput")

    xqT = din("xqT", (D, S), bf16)
    xkT = din("xkT", (D, S), bf16)
    xvT = din("xvT", (D, S), bf16)
    wqT = din("wqT", (D, D), bf16)
    wkT = din("wkT", (D, D), bf16)
    wvT = din("wvT", (D, D), bf16)
    woT = din("woT", (D, D), bf16)
    bqk = din("bqk", (128, 16), fp32)        # cols 0-7: bq chunks, 8-15: bk chunks
    bvrow = din("bvrow", (1, D), bf16)
    ones1 = din("ones1", (1, 128), bf16)
    negg = din("negg", (128, H + 2), fp32)   # -softplus(gamma)*2^(0x1FBC/128); col H = ln(1/16), col H+1 = ln(1/8)
    posflip = din("posflip", (128, MEGA), fp16)  # max((128*qb+p) - k, 0) per mega segment
    rstrev = din("rstrev", (128, MEGA), fp16)  # 0.0 at per-row diagonal (in
    # reversed traversal order: col t maps to mega col MEGA-1-t) else 1.0
    negtri = din("negtri", (128, 128), bf16)   # j>p -> -1e30 else 0
    ident = din("ident", (128, 128), bf16)

    out_d = nc.dram_tensor("out", [S, D], fp32, kind="ExternalOutput")

    with tile.TileContext(nc) as tc:
        with tc.tile_pool(name="persist", bufs=1) as pp, \
             tc.tile_pool(name="mmps", bufs=2, space="PSUM") as mmps, \
             tc.tile_pool(name="trps", bufs=2, space="PSUM") as trps:
            qmT = [pp.tile([128, S], bf16, tag=f"qmT{c}", name=f"qmT{c}")
                   for c in range(8)]
            kmT = [pp.tile([128, S], bf16, tag=f"kmT{c}", name=f"kmT{c}")
                   for c in range(8)]
            vm = [pp.tile([128, D], bf16, tag=f"vm{c}", name=f"vm{c}")
                  for c in range(8)]
            omT = [pp.tile([128, S], bf16, tag=f"omT{c}", name=f"omT{c}")
                   for c in range(8)]
            bqk_sb = pp.tile([128, 16], fp32, tag="bqk")
            negg_sb = pp.tile([128, H + 2], fp32, tag="negg")
            negtri_sb = pp.tile([128, 128], bf16, tag="negtri")
            ident_sb = pp.tile([128, 128], bf16, tag="ident")
            bv_sb = pp.tile([1, D], bf16, tag="bv")
            ones1_sb = pp.tile([1, 128], bf16, tag="ones1")

            nc.sync.dma_start(out=bqk_sb[:], in_=bqk.ap()[:, :])
            nc.sync.dma_start(out=negg_sb[:], in_=negg.ap()[:, :])
            nc.sync.dma_start(out=negtri_sb[:], in_=negtri.ap()[:, :])
            nc.sync.dma_start(out=ident_sb[:], in_=ident.ap()[:, :])
            nc.sync.dma_start(out=bv_sb[:], in_=bvrow.ap()[:, :])
            nc.sync.dma_start(out=ones1_sb[:], in_=ones1.ap()[:, :])

            def proj_qk(wh, xh, dest, bcol, wtag):
                """One transposed projection (q or k) in its own SBUF scope.
                PSUM drain + bias add on the Pool engine (ACT is exp-bound)."""
                with tc.tile_pool(name=f"p1{wtag}", bufs=1) as sp1:
                    w_t = [sp1.tile([128, D], bf16, tag=f"w{wtag}{c}",
                                    name=f"w{wtag}{c}") for c in range(8)]
                    x_t = [sp1.tile([128, S], bf16, tag=f"x{wtag}{c}",
                                    name=f"x{wtag}{c}") for c in range(8)]
                    for c in range(8):
                        r = slice(c * 128, (c + 1) * 128)
                        nc.sync.dma_start(out=w_t[c][:], in_=wh.ap()[r, :])
                        nc.sync.dma_start(out=x_t[c][:], in_=xh.ap()[r, :])
                    for dc in range(8):
                        ps = mmps.tile([128, S], fp32, tag="mm",
                                       name=f"ps{wtag}{dc}")
                        for c in range(8):
                            for j in range(0, S, 512):
                                nc.tensor.matmul(
                                    out=ps[:, j:j + 512],
                                    lhsT=w_t[c][:, dc * 128:(dc + 1) * 128],
                                    rhs=x_t[c][:, j:j + 512],
                                    start=(c == 0), stop=(c == 7),
                                )
                        nc.scalar.activation(
                            out=dest[dc][:], in_=ps[:],
                            func=mybir.ActivationFunctionType.Identity,
                            bias=bqk_sb[:, bcol + dc:bcol + dc + 1], scale=1.0)

            def proj_v():
                # x streamed in half-width tiles to keep this scope small
                # while attention pair 0 is in flight.
                with tc.tile_pool(name="p1v", bufs=1) as sp1, \
                     tc.tile_pool(name="p1vx", bufs=2) as sx:
                    w_t = [sp1.tile([128, D], bf16, tag=f"wv{c}",
                                    name=f"wv{c}") for c in range(8)]
                    for c in range(8):
                        nc.sync.dma_start(out=w_t[c][:],
                                          in_=wvT.ap()[c * 128:(c + 1) * 128, :])
                    for quar in range(4):
                        x_t = [sx.tile([128, S // 4], bf16, tag=f"xv{c}",
                                       name=f"xv{quar}_{c}") for c in range(8)]
                        for c in range(8):
                            nc.sync.dma_start(
                                out=x_t[c][:],
                                in_=xvT.ap()[c * 128:(c + 1) * 128,
                                             quar * 256:(quar + 1) * 256])
                        for sbh in range(2):
                            sb = quar * 2 + sbh
                            ps = mmps.tile([128, D], fp32, tag="mm",
                                           name=f"psv{sb}")
                            for j in range(0, D, 512):
                                for c in range(8):
                                    nc.tensor.matmul(
                                        out=ps[:, j:j + 512],
                                        lhsT=x_t[c][:, sbh * 128:(sbh + 1) * 128],
                                        rhs=w_t[c][:, j:j + 512],
                                        start=(c == 0), stop=False,
                                    )
                                nc.tensor.matmul(
                                    out=ps[:, j:j + 512],
                                    lhsT=ones1_sb[:, :],
                                    rhs=bv_sb[:, j:j + 512],
                                    start=False, stop=True,
                                )
                            nc.scalar.copy(out=vm[sb][:], in_=ps[:])

            proj_qk(wqT, xqT, qmT, 0, "q")
            proj_qk(wkT, xkT, kmT, 8, "k")

            if not do_attn:
                proj_v()
                state = {}
            else:
              with tc.tile_pool(name="p2c", bufs=1) as cp, \
                   tc.tile_pool(name="p2f", bufs=2) as mf, \
                   tc.tile_pool(name="p2b", bufs=6) as mb, \
                   tc.tile_pool(name="p2m", bufs=2) as e2p, \
                   tc.tile_pool(name="p2s", bufs=2) as sstats, \
                   tc.tile_pool(name="p2e", bufs=4) as ep, \
                   tc.tile_pool(name="omps", bufs=1, space="PSUM") as omps:
                posflip_sb = cp.tile([128, MEGA], fp16, tag="posflip")
                rstrev_sb = cp.tile([128, MEGA], fp16, tag="rstrev")
                nc.sync.dma_start(out=posflip_sb[:], in_=posflip.ap()[:, :])
                nc.sync.dma_start(out=rstrev_sb[:], in_=rstrev.ap()[:, :])

                state = {}

                def emit_A(hp):
                    """Pass A: e1 = exp(s/8)/16 (unmasked), reversed
                    per-row-reset scan in place -> inclusive suffix sums
                    (value at each segment start = masked row total), then
                    u = strict_suffix*pos/tot (strict = read at +1),
                    bit-shift sqrt, eff' = exp(g*dist)/8. Per-row diagonal
                    reset makes pass-A causal masking unnecessary."""
                    st = {"e1m": {}, "u": {}, "eff": {}, "rtots": {},
                          "srgh": {}}
                    state[hp] = st
                    for hh in range(2):
                        h = 2 * hp + hh
                        hr = slice(hh * 64, (hh + 1) * 64)
                        e1m = mf.tile([128, MEGA + 1], fp16, tag="megah",
                                      name=f"e1m{h}")
                        st["e1m"][hh] = e1m
                        nc.gpsimd.memset(e1m[:, MEGA:MEGA + 1], 0.0)
                        for qb in range(NT):
                            W = 128 * (qb + 1)
                            off = OFFS[qb]
                            sp = mmps.tile([128, S], fp32, tag="mm",
                                           name=f"spA{h}_{qb}")
                            for j in range(0, W, 512):
                                je = min(j + 512, W)
                                nc.tensor.matmul(
                                    out=sp[:, j:je],
                                    lhsT=qmT[hp][hr, qb * 128:(qb + 1) * 128],
                                    rhs=kmT[hp][hr, j:je],
                                    start=True, stop=True,
                                )
                            nc.scalar.activation(
                                out=e1m[:, off:off + W], in_=sp[:, :W],
                                func=mybir.ActivationFunctionType.Exp,
                                scale=0.125, bias=negg_sb[:, H:H + 1])
                    for hh in range(2):
                        st_e1m = st["e1m"][hh]
                        ranges = [(0, MEGA, nc.vector)]
                        if scan_split:
                            eng = (nc.gpsimd if scan_split_engine == "gpsimd"
                                   else nc.vector)
                            ranges = [(0, scan_split, nc.vector),
                                      (scan_split, MEGA, eng)]
                        for (a, b, eng) in ranges:
                            fwd = st_e1m[:, 0:MEGA]
                            rev = bass.AP(fwd.tensor, fwd.offset + b - 1,
                                          [list(fwd.ap[0]), [-1, b - a]])
                            eng.tensor_tensor_scan(
                                out=rev,
                                data0=rstrev_sb[:, MEGA - b:MEGA - a],
                                data1=rev, initial=0.0,
                                op0=mybir.AluOpType.mult,
                                op1=mybir.AluOpType.add)
                    # Per-head decay scale: srgh = rsqrt(tot) * (-g*cm), one
                    # value per (partition=q, qb). Seeded by the fp32
                    # bit-shift sqrt of rtots, refined with one Newton
                    # iteration (y1 = y0*(1.5 - 0.5*tot*y0^2)).
                    for hh in range(2):
                        h = 2 * hp + hh
                        rtots = sstats.tile([128, 5 * NT], fp32, tag="rtots",
                                            name=f"rtots{h}")
                        st["rtots"][hh] = rtots
                        rt = rtots[:, 0:NT]
                        tot8 = rtots[:, NT:2 * NT]
                        ysq = rtots[:, 2 * NT:3 * NT]
                        ycor = rtots[:, 3 * NT:4 * NT]
                        srgh = rtots[:, 4 * NT:5 * NT]
                        st["srgh"][hh] = srgh
                        for qb in range(NT):
                            off = OFFS[qb]
                            nc.vector.reciprocal(
                                out=rt[:, qb:qb + 1],
                                in_=st["e1m"][hh][:, off:off + 1])
                        nc.vector.reciprocal(out=tot8, in_=rt)
                        y0u = rt.bitcast(mybir.dt.uint32)
                        nc.vector.tensor_scalar(
                            out=y0u, in0=y0u, scalar1=1, scalar2=None,
                            op0=mybir.AluOpType.logical_shift_right,
                            op1=mybir.AluOpType.bypass)
                        nc.vector.tensor_scalar(
                            out=y0u, in0=y0u, scalar1=0x1FBC0000,
                            scalar2=None,
                            op0=mybir.AluOpType.add,
                            op1=mybir.AluOpType.bypass)
                        nc.vector.tensor_tensor(
                            out=ysq, in0=rt, in1=rt, op=mybir.AluOpType.mult)
                        nc.vector.tensor_tensor(
                            out=ysq, in0=ysq, in1=tot8,
                            op=mybir.AluOpType.mult)
                        nc.vector.tensor_scalar(
                            out=ycor, in0=ysq, scalar1=-0.5, scalar2=1.5,
                            op0=mybir.AluOpType.mult,
                            op1=mybir.AluOpType.add)
                        nc.vector.tensor_tensor(
                            out=ycor, in0=ycor, in1=rt,
                            op=mybir.AluOpType.mult)
                        nc.vector.tensor_scalar(
                            out=srgh, in0=ycor,
                            scalar1=negg_sb[:, h:h + 1], scalar2=None,
                            op0=mybir.AluOpType.mult,
                            op1=mybir.AluOpType.bypass)
                    for hh in range(2):
                        h = 2 * hp + hh
                        u = mb.tile([128, MEGA], bf16,
                                    tag="megab", name=f"u{h}")
                        st["u"][hh] = u
                        u0_eng = nc.gpsimd if pool_u0 else nc.vector
                        u0_eng.tensor_tensor(
                            out=u[:], in0=st["e1m"][hh][:, 1:MEGA + 1],
                            in1=posflip_sb[:], op=mybir.AluOpType.mult)
                    for hh in range(2):
                        uv = st["u"][hh][:].bitcast(mybir.dt.uint16)
                        eng = nc.gpsimd if pool_shift else nc.vector
                        eng.tensor_scalar(
                            out=uv, in0=uv, scalar1=1, scalar2=None,
                            op0=mybir.AluOpType.logical_shift_right,
                            op1=mybir.AluOpType.bypass)
                    for hh in range(2):
                        h = 2 * hp + hh
                        eff = mb.tile([128, MEGA], bf16, tag="megab",
                                      name=f"eff{h}")
                        st["eff"][hh] = eff
                        for qb in range(NT):
                            W = 128 * (qb + 1)
                            off = OFFS[qb]
                            nc.scalar.activation(
                                out=eff[:, off:off + W],
                                in_=st["u"][hh][:, off:off + W],
                                func=mybir.ActivationFunctionType.Exp,
                                scale=st["srgh"][hh][:, qb:qb + 1],
                                bias=negg_sb[:, H + 1:H + 2])

                def emit_B(hp):
                    st = state.pop(hp)
                    om_ps = omps.tile([128, S], fp32, tag="om",
                                      name=f"om_ps{hp}")
                    e2m = {}
                    tot2s = {}
                    for hh in range(2):
                        h = 2 * hp + hh
                        hr = slice(hh * 64, (hh + 1) * 64)
                        e2m[hh] = e2p.tile([128, MEGA], bf16, tag="e2mega",
                                           name=f"e2m{h}")
                        tot2s[hh] = sstats.tile([128, NT], fp32, tag="tot2s",
                                                name=f"tot2s{h}")
                        for qb in range(NT):
                            W = 128 * (qb + 1)
                            off = OFFS[qb]
                            sp = mmps.tile([128, S], fp32, tag="mm",
                                           name=f"spB{h}_{qb}")
                            for j in range(0, W, 512):
                                je = min(j + 512, W)
                                nc.tensor.matmul(
                                    out=sp[:, j:je],
                                    lhsT=qmT[hp][hr, qb * 128:(qb + 1) * 128],
                                    rhs=kmT[hp][hr, j:je],
                                    start=True, stop=True,
                                )
                            # diagonal-block causal mask folded into the PE:
                            # I @ negtri accumulates -1e30 above the diagonal
                            # (eff there is exactly 1/8, so it stays -1e29
                            # after the s2 multiply).
                            nc.tensor.matmul(
                                out=sp[:, W - 128:W],
                                lhsT=ident_sb[:],
                                rhs=negtri_sb[:],
                                start=False, stop=True,
                                skip_group_check=True,
                            )
                            # s2 = s * eff in place in PSUM, then exp
                            nc.vector.tensor_tensor(
                                out=sp[:, :W], in0=sp[:, :W],
                                in1=st["eff"][hh][:, off:off + W],
                                op=mybir.AluOpType.mult)
                            nc.scalar.activation(
                                out=e2m[hh][:, off:off + W],
                                in_=sp[:, :W],
                                func=mybir.ActivationFunctionType.Exp,
                                accum_out=tot2s[hh][:, qb:qb + 1])
                    for hh in range(2):
                        h = 2 * hp + hh
                        hr = slice(hh * 64, (hh + 1) * 64)
                        rt2 = sstats.tile([128, NT], fp32, tag="rt2",
                                          name=f"rt2_{h}")
                        nc.vector.reciprocal(out=rt2[:], in_=tot2s[hh][:])
                        norm_eng = nc.gpsimd if pool_norm else nc.vector
                        for qb in range(NT):
                            W = 128 * (qb + 1)
                            off = OFFS[qb]
                            norm_eng.tensor_scalar_mul(
                                out=e2m[hh][:, off:off + W],
                                in0=e2m[hh][:, off:off + W],
                                scalar1=rt2[:, qb:qb + 1])
                        # PE-transpose each 128-col block of e2 into PSUM
                        # (bf16), drain on DVE, then P^T V accumulation.
                        # PV for qb is emitted after the transposes of qb+1
                        # so the PE is not stalled on the qb drain.
                        e2T = {}
                        prev = None

                        def emit_pv(qb):
                            for c in range(qb + 1):
                                nc.tensor.matmul(
                                    out=om_ps[hr, qb * 128:(qb + 1) * 128],
                                    lhsT=vm[c][:, h * 64:(h + 1) * 64],
                                    rhs=e2T[qb][:, c * 128:(c + 1) * 128],
                                    start=(c == 0), stop=(c == qb),
                                )

                        for qb in range(NT):
                            W = 128 * (qb + 1)
                            off = OFFS[qb]
                            e2T[qb] = ep.tile([128, S], bf16, tag="e2T",
                                              name=f"e2T{h}_{qb}")
                            for g0 in range(0, qb + 1, 4):
                                g1 = min(g0 + 4, qb + 1)
                                trp = trps.tile([128, 512], bf16, tag="tr",
                                                name=f"tr{h}_{qb}_{g0}")
                                for c in range(g0, g1):
                                    nc.tensor.transpose(
                                        trp[:, (c - g0) * 128:(c - g0 + 1) * 128],
                                        e2m[hh][:, off + c * 128:off + (c + 1) * 128],
                                        ident_sb[:])
                                nc.vector.tensor_copy(
                                    out=e2T[qb][:, g0 * 128:g1 * 128],
                                    in_=trp[:, 0:(g1 - g0) * 128])
                            if prev is not None:
                                emit_pv(prev)
                            prev = qb
                        emit_pv(prev)
                    nc.scalar.copy(out=omT[hp][:], in_=om_ps[:])

                # Software pipeline: A(hp+1) is emitted before B(hp) so the
                # PE has independent matmul work while pair hp's scan/decay
                # chain runs on DVE/Pool/ACT.
                emit_A(0)
                if n_hp > 1:
                    emit_A(1)
                proj_v()
                for hp in range(n_hp):
                    emit_B(hp)
                    if hp + 2 < n_hp:
                        emit_A(hp + 2)

            # ================= phase 3: output projection =================
            if not do_out:
                for name in dbg_dump:
                    t = {**{f"qmT{c}": qmT[c] for c in range(8)},
                         **{f"kmT{c}": kmT[c] for c in range(8)},
                         **{f"vm{c}": vm[c] for c in range(8)},
                         **{f"omT{c}": omT[c] for c in range(8)}}[name]
                    dd = nc.dram_tensor(f"dbg_{name}", list(t.shape),
                                        t.dtype, kind="ExternalOutput")
                    nc.sync.dma_start(out=dd.ap()[:, :], in_=t[:])
            else:
                with tc.tile_pool(name="p3w", bufs=1) as wop, \
                     tc.tile_pool(name="p3o", bufs=2) as outp:
                    wo_t = [wop.tile([128, D], bf16, tag=f"wo{c}", name=f"wo{c}")
                            for c in range(8)]
                    for c in range(8):
                        nc.sync.dma_start(out=wo_t[c][:],
                                          in_=woT.ap()[c * 128:(c + 1) * 128, :])
                    for sb in range(8):
                        ps = mmps.tile([128, D], fp32, tag="mm", name=f"ps3{sb}")
                        for j in range(0, D, 512):
                            for c in range(8):
                                nc.tensor.matmul(
                                    out=ps[:, j:j + 512],
                                    lhsT=omT[c][:, sb * 128:(sb + 1) * 128],
                                    rhs=wo_t[c][:, j:j + 512],
                                    start=(c == 0), stop=(c == 7),
                                )
                        st = outp.tile([128, D], fp32, tag="ost", name=f"ost{sb}")
                        nc.scalar.copy(out=st[:], in_=ps[:])
                        nc.sync.dma_start(out=out_d.ap()[sb * 128:(sb + 1) * 128, :],
                                          in_=st[:])
    nc.compile()
    return nc


def _host_constants():
    p = np.arange(128, dtype=np.int64)[:, None]
    posflip = np.zeros((128, MEGA), np.float32)
    rstrev = np.ones((128, MEGA), np.float32)
    for qb in range(NT):
        W = 128 * (qb + 1)
        off = OFFS[qb]
        k = np.arange(W, dtype=np.int64)[None, :]
        # clamp to >=0: above the diagonal (masked region, incl. the one
        # cross-segment suffix read at k=W-1) u becomes exactly +0.
        posflip[:, off:off + W] = np.maximum(
            (128 * qb + p) - k, 0).astype(np.float32)
        # inclusive scan with per-row reset AT the diagonal: the scan value
        # there restarts from e1[diag], so segment-start values are masked
        # row totals and garbage above the diagonal never crosses.
        for pp_ in range(128):
            rstrev[pp_, MEGA - 1 - (off + 128 * qb + pp_)] = 0.0
    jj = np.arange(128)[None, :]
    negtri = np.where(jj > p, -1e30, 0.0).astype(BF16)
    ident = np.eye(128, dtype=np.float32)
    return (posflip.astype(np.float16), rstrev.astype(np.float16), negtri,
            ident.astype(BF16))


def _softplus(x):
    return np.log1p(np.exp(-np.abs(x))) + np.maximum(x, 0.0)


def _make_in_maps(inputs):
    posflip, rstrev, negtri, ident = _host_constants()
    g = _softplus(np.asarray(inputs["gammas"], np.float32).reshape(H))
    cm = 2.0 ** (0x1FBC / 128.0)  # bf16 shift-sqrt correction
    negg = np.zeros((128, H + 2), np.float32)
    negg[:, :H] = -g[None, :] * cm
    negg[:, H] = LN1_16
    negg[:, H + 1] = np.log(1.0 / 8.0)
    ones1 = np.ones((1, 128), BF16)

    in_maps = []
    for t, tag in ((0, "m"), (1, "c")):
        key = "mean" if t == 0 else "cov"
        wq = np.asarray(inputs[f"wq_{tag}"], np.float32)
        wk = np.asarray(inputs[f"wk_{tag}"], np.float32)
        wv = np.asarray(inputs[f"wv_{tag}"], np.float32)
        wo = np.asarray(inputs[f"wo_{tag}"], np.float32)
        bq = np.asarray(inputs[f"bq_{tag}"], np.float32)
        bk = np.asarray(inputs[f"bk_{tag}"], np.float32)
        bv = np.asarray(inputs[f"bv_{tag}"], np.float32)
        bqk = np.concatenate([bq.reshape(8, 128).T, bk.reshape(8, 128).T],
                             axis=1).astype(np.float32)
        wqT = np.ascontiguousarray(wq.T).astype(BF16)
        wkT = np.ascontiguousarray(wk.T).astype(BF16)
        wvT = np.ascontiguousarray(wv.T).astype(BF16)
        woT = np.ascontiguousarray(wo.T).astype(BF16)
        bvrow = bv.reshape(1, D).astype(BF16)
        for b in range(B):
            xq = np.asarray(inputs[f"query_{key}"][b], np.float32)
            xk = np.asarray(inputs[f"key_{key}"][b], np.float32)
            xv = np.asarray(inputs[f"values_{key}"][b], np.float32)
            in_maps.append({
                "xqT": np.ascontiguousarray(xq.T).astype(BF16),
                "xkT": np.ascontiguousarray(xk.T).astype(BF16),
                "xvT": np.ascontiguousarray(xv.T).astype(BF16),
                "wqT": wqT, "wkT": wkT, "wvT": wvT, "woT": woT,
                "bqk": bqk, "bvrow": bvrow, "ones1": ones1,
                "negg": negg, "posflip": posflip, "rstrev": rstrev,
                "negtri": negtri, "ident": ident,
            })
    return in_maps


def _numpy_reference(inputs):
    """Pure-numpy fallback replicating reference.py (used only if the mask is
    not causal-tril, which the staged problem never produces)."""
    def lin(x, w, b):
        return x @ w.T + b

    def split_heads(x):
        return x.reshape(B, S, H, DK).transpose(0, 2, 1, 3)

    def merge_heads(x):
        return x.transpose(0, 2, 1, 3).reshape(B, S, D)

    def softmax(x):
        m = x.max(-1, keepdims=True)
        e = np.exp(x - m)
        return e / e.sum(-1, keepdims=True)

    mask_f = np.asarray(inputs["mask"], np.float32)
    idx = np.arange(S, dtype=np.float32)
    pos = np.abs(idx[None, :] - idx[:, None])
    g = -_softplus(np.asarray(inputs["gammas"], np.float32))[None]

    outs = []
    for tag, key in (("m", "mean"), ("c", "cov")):
        q = split_heads(lin(np.asarray(inputs[f"query_{key}"], np.float32),
                            np.asarray(inputs[f"wq_{tag}"]), np.asarray(inputs[f"bq_{tag}"])))
        k = split_heads(lin(np.asarray(inputs[f"key_{key}"], np.float32),
                            np.asarray(inputs[f"wk_{tag}"]), np.asarray(inputs[f"bk_{tag}"])))
        v = split_heads(lin(np.asarray(inputs[f"values_{key}"], np.float32),
                            np.asarray(inputs[f"wv_{tag}"]), np.asarray(inputs[f"bv_{tag}"])))
        sc = np.einsum('bhqd,bhkd->bhqk', q, k) / np.sqrt(DK)
        pm = softmax(np.where(mask_f == 0, -1e32, sc)) * mask_f
        cum = np.cumsum(pm, -1)
        tot = pm.sum(-1, keepdims=True)
        dist = np.sqrt(np.clip((tot - cum) * pos, 0.0, None))
        effd = np.clip(np.exp(dist * g), 1e-5, 1e5)
        s = np.where(mask_f == 0, -1e32, sc * effd)
        p = softmax(s)
        o = merge_heads(np.einsum('bhqk,bhkd->bhqd', p, v))
        outs.append(lin(o, np.asarray(inputs[f"wo_{tag}"]),
                        np.asarray(inputs[f"bo_{tag}"])))
    return np.stack(outs, 0).astype(np.float32)


def _ensure_ntff_hook():
    """The container's antenv lacks axon_hooks; bass_utils imports it
    unguarded when trace=True. Provide it, backed by the ctypes NTFF
    profiler from trn_agent_boot when available."""
    import sys as _sys
    import types as _types
    if "antenv.axon_hooks" in _sys.modules:
        return
    hook = None
    try:
        from trn_agent_boot.trn_boot import _ntff_profile_via_ctypes
        hook = _ntff_profile_via_ctypes("/opt/axon/libaxon_pjrt.so")
    except Exception:
        hook = None
    mod = _types.ModuleType("antenv.axon_hooks")
    mod.get_axon_ntff_profile_hook = lambda: hook
    mod.set_axon_ntff_profile_hook = lambda h: None
    _sys.modules["antenv.axon_hooks"] = mod
    try:
        import antenv
        antenv.axon_hooks = mod
    except Exception:
        pass


def run(inputs, trace=False):
    if trace:
        _ensure_ntff_hook()
    mask = np.asarray(inputs["mask"]).reshape(S, S)
    if not np.array_equal(mask, np.tril(np.ones((S, S), mask.dtype))):
        return _numpy_reference(inputs), None

    if "nc" not in _cache:
        _cache["nc"] = build_bass()
    nc = _cache["nc"]
    in_maps = _make_in_maps(inputs)
    res = run_bass_kernel_spmd(nc, in_maps, core_ids=list(range(8)), trace=trace)

    bo_m = np.asarray(inputs["bo_m"], np.float32)
    bo_c = np.asarray(inputs["bo_c"], np.float32)
    out = np.zeros((2, B, S, D), np.float32)
    for t in range(2):
        bo = bo_m if t == 0 else bo_c
        for b in range(B):
            out[t, b] = res.results[t * 4 + b]["out"] + bo[None, :]
    return out, res


def kernel(**inputs) -> np.ndarray:
    out, _ = run(inputs, trace=False)
    return out


# revision 23
# speedup vs baseline: 1.1487x; 1.1487x over previous
"""Trainium2 Bass kernel for nn_Architecture_79353815760942 (decay-attention
dense transformer, B=4 S=1024 D=1024 H=16, mean+cov twin pipelines).

Sharding: 8 cores = 2 tensor-types (mean, cov) x 4 batches. The mean and cov
pipelines are fully independent, so each core runs one (type, batch) slice
end-to-end with zero inter-core communication.

Per-core pipeline (all layouts chosen so no on-chip transposes of activations
are needed except the attention-probability tiles, which go through the PE
transpose unit):
  phase 1: q/k/v projections from host-pre-transposed bf16 inputs
           qmT,kmT in [d_head, seq] layout; v in [seq, d_head] layout
  phase 2: per head: scores -> exp -> segmented-scan cumsum -> decay factor
           (via ln/exp to stay in one ACT table set) -> second softmax ->
           PE-transpose -> P^T V accumulation (omT layout)
  phase 3: output projection from omT, f32 DMA out.

Engine budget (per core, measured-calibrated):
  PE: projections + 2x scores + PV + PE-transposes, emitted interleaved
      across head pairs so the tensor engine never idles (p-state ramp).
  ACT: the three exp passes.
  DVE: scan, s2-mult (in-psum), shift-sqrt, e2 normalize, psum drains.
  Pool: u-step STT, diagonal mask adds, projection psum drains.
"""
import os
import numpy as np
import ml_dtypes

from concourse import bass, bacc, tile, mybir
from concourse.bass_utils import run_bass_kernel_spmd

BF16 = ml_dtypes.bfloat16
F32 = np.float32

B, S, D, H = 4, 1024, 1024, 16
DK = D // H          # 64
NT = S // 128        # 8 query tiles per (b,h)
OFFS = [128 * qb * (qb + 1) // 2 for qb in range(NT)]   # mega col offset per q tile
MEGA = OFFS[-1] + 128 * NT  # 4608
NEG = -1e30
LN1_16 = float(np.log(1.0 / 16.0))  # exp1 bias: keeps e1/suffix sums in fp16 range

_cache = {}


def _ceil_div(a, b):
    return (a + b - 1) // b


def build_bass(do_attn=True, do_out=True, n_hp=8, dbg_dump=(),
               scan_split=None, scan_split_engine="vector",
               pool_shift=False, pool_norm=False, pool_u0=True):
    """Build the SPMD single-core program (same graph on all 8 cores).

    Emission order software-pipelines the head pairs: pass A of pair hp+1 is
    emitted before pass B of pair hp, so the tensor engine always has matmul
    work while pair hp's scan/decay chain runs on DVE/Pool/ACT.

    scan_split: optional mega-col tile boundary (an OFFS value); the scan
    for cols [split, MEGA) runs on scan_split_engine instead of DVE. The
    per-row diagonal resets make tile segments independent, so any tile
    boundary is a safe split point.
    """
    fp32 = mybir.dt.float32
    bf16 = mybir.dt.bfloat16
    fp16 = mybir.dt.float16

    nc = bacc.Bacc("TRN2", target_bir_lowering=False, debug=False, num_devices=8)

    def din(name, shape, dt):
        return nc.dram_tensor(name, list(shape), dt, kind="ExternalInput")

    xqT = din("xqT", (D, S), bf16)
    xkT = din("xkT", (D, S), bf16)
    xvT = din("xvT", (D, S), bf16)
    wqT = din("wqT", (D, D), bf16)
    wkT = din("wkT", (D, D), bf16)
    wvT = din("wvT", (D, D), bf16)
    woT = din("woT", (D, D), bf16)
    bqk = din("bqk", (128, 16), fp32)        # cols 0-7: bq chunks, 8-15: bk chunks
    bvrow = din("bvrow", (1, D), bf16)
    ones1 = din("ones1", (1, 128), bf16)
    negg = din("negg", (128, H + 2), fp32)   # -softplus(gamma)*2^(0x1FBC/128); col H = ln(1/16), col H+1 = ln(1/8)
    posflip = din("posflip", (128, MEGA), fp16)  # max((128*qb+p) - k, 0) per mega segment
    rstrev = din("rstrev", (128, MEGA), fp16)  # 0.0 at per-row diagonal (in
    # reversed traversal order: col t maps to mega col MEGA-1-t) else 1.0
    negtri = din("negtri", (128, 128), bf16)   # j>p -> -1e30 else 0
    ident = din("ident", (128, 128), bf16)

    out_d = nc.dram_tensor("out", [S, D], fp32, kind="ExternalOutput")

    with tile.TileContext(nc) as tc:
        with tc.tile_pool(name="persist", bufs=1) as pp, \
             tc.tile_pool(name="mmps", bufs=3, space="PSUM") as mmps:
            qmT = [pp.tile([128, S], bf16, tag=f"qmT{c}", name=f"qmT{c}")
                   for c in range(8)]
            kmT = [pp.tile([128, S], bf16, tag=f"kmT{c}", name=f"kmT{c}")
                   for c in range(8)]
            vm = [pp.tile([128, D], bf16, tag=f"vm{c}", name=f"vm{c}")
                  for c in range(8)]
            omT = [pp.tile([128, S], bf16, tag=f"omT{c}", name=f"omT{c}")
                   for c in range(8)]
            bqk_sb = pp.tile([128, 16], fp32, tag="bqk")
            negg_sb = pp.tile([128, H + 2], fp32, tag="negg")
            negtri_sb = pp.tile([128, 128], bf16, tag="negtri")
            ident_sb = pp.tile([128, 128], bf16, tag="ident")
            bv_sb = pp.tile([1, D], bf16, tag="bv")
            ones1_sb = pp.tile([1, 128], bf16, tag="ones1")

            nc.sync.dma_start(out=bqk_sb[:], in_=bqk.ap()[:, :])
            nc.sync.dma_start(out=negg_sb[:], in_=negg.ap()[:, :])
            nc.sync.dma_start(out=negtri_sb[:], in_=negtri.ap()[:, :])
            nc.sync.dma_start(out=ident_sb[:], in_=ident.ap()[:, :])
            nc.sync.dma_start(out=bv_sb[:], in_=bvrow.ap()[:, :])
            nc.sync.dma_start(out=ones1_sb[:], in_=ones1.ap()[:, :])

            def proj_qk(wh, xh, dest, bcol, wtag):
                """One transposed projection (q or k) in its own SBUF scope.
                PSUM drain + bias add on the Pool engine (ACT is exp-bound)."""
                with tc.tile_pool(name=f"p1{wtag}", bufs=1) as sp1:
                    w_t = [sp1.tile([128, D], bf16, tag=f"w{wtag}{c}",
                                    name=f"w{wtag}{c}") for c in range(8)]
                    x_t = [sp1.tile([128, S], bf16, tag=f"x{wtag}{c}",
                                    name=f"x{wtag}{c}") for c in range(8)]
                    for c in range(8):
                        r = slice(c * 128, (c + 1) * 128)
                        nc.sync.dma_start(out=w_t[c][:], in_=wh.ap()[r, :])
                        nc.sync.dma_start(out=x_t[c][:], in_=xh.ap()[r, :])
                    for dc in range(8):
                        ps = mmps.tile([128, S], fp32, tag="mm",
                                       name=f"ps{wtag}{dc}")
                        for c in range(8):
                            for j in range(0, S, 512):
                                nc.tensor.matmul(
                                    out=ps[:, j:j + 512],
                                    lhsT=w_t[c][:, dc * 128:(dc + 1) * 128],
                                    rhs=x_t[c][:, j:j + 512],
                                    start=(c == 0), stop=(c == 7),
                                )
                        nc.scalar.activation(
                            out=dest[dc][:], in_=ps[:],
                            func=mybir.ActivationFunctionType.Identity,
                            bias=bqk_sb[:, bcol + dc:bcol + dc + 1], scale=1.0)

            def proj_v():
                # x streamed in half-width tiles to keep this scope small
                # while attention pair 0 is in flight.
                with tc.tile_pool(name="p1v", bufs=1) as sp1, \
                     tc.tile_pool(name="p1vx", bufs=2) as sx:
                    w_t = [sp1.tile([128, D], bf16, tag=f"wv{c}",
                                    name=f"wv{c}") for c in range(8)]
                    for c in range(8):
                        nc.sync.dma_start(out=w_t[c][:],
                                          in_=wvT.ap()[c * 128:(c + 1) * 128, :])
                    for quar in range(4):
                        x_t = [sx.tile([128, S // 4], bf16, tag=f"xv{c}",
                                       name=f"xv{quar}_{c}") for c in range(8)]
                        for c in range(8):
                            nc.sync.dma_start(
                                out=x_t[c][:],
                                in_=xvT.ap()[c * 128:(c + 1) * 128,
                                             quar * 256:(quar + 1) * 256])
                        for sbh in range(2):
                            sb = quar * 2 + sbh
                            ps = mmps.tile([128, D], fp32, tag="mm",
                                           name=f"psv{sb}")
                            for j in range(0, D, 512):
                                for c in range(8):
                                    nc.tensor.matmul(
                                        out=ps[:, j:j + 512],
                                        lhsT=x_t[c][:, sbh * 128:(sbh + 1) * 128],
                                        rhs=w_t[c][:, j:j + 512],
                                        start=(c == 0), stop=False,
                                    )
                                nc.tensor.matmul(
                                    out=ps[:, j:j + 512],
                                    lhsT=ones1_sb[:, :],
                                    rhs=bv_sb[:, j:j + 512],
                                    start=False, stop=True,
                                )
                            nc.scalar.copy(out=vm[sb][:], in_=ps[:])

            proj_qk(wqT, xqT, qmT, 0, "q")
            proj_qk(wkT, xkT, kmT, 8, "k")

            if not do_attn:
                proj_v()
                state = {}
            else:
              with tc.tile_pool(name="p2c", bufs=1) as cp, \
                   tc.tile_pool(name="p2f", bufs=2) as mf, \
                   tc.tile_pool(name="p2b", bufs=5) as mb, \
                   tc.tile_pool(name="p2m", bufs=2) as e2p, \
                   tc.tile_pool(name="p2s", bufs=2) as sstats, \
                   tc.tile_pool(name="p2e", bufs=1) as ep, \
                   tc.tile_pool(name="omps", bufs=1, space="PSUM") as omps:
                posflip_sb = cp.tile([128, MEGA], fp16, tag="posflip")
                rstrev_sb = cp.tile([128, MEGA], fp16, tag="rstrev")
                nc.sync.dma_start(out=posflip_sb[:], in_=posflip.ap()[:, :])
                nc.sync.dma_start(out=rstrev_sb[:], in_=rstrev.ap()[:, :])

                state = {}

                def emit_A(hp):
                    """Pass A: e1 = exp(s/8)/16 (unmasked), reversed
                    per-row-reset scan in place -> inclusive suffix sums
                    (value at each segment start = masked row total), then
                    u = strict_suffix*pos/tot (strict = read at +1),
                    bit-shift sqrt, eff' = exp(g*dist)/8. Per-row diagonal
                    reset makes pass-A causal masking unnecessary."""
                    st = {"e1m": {}, "u": {}, "eff": {}, "rtots": {},
                          "srgh": {}}
                    state[hp] = st
                    for hh in range(2):
                        h = 2 * hp + hh
                        hr = slice(hh * 64, (hh + 1) * 64)
                        e1m = mf.tile([128, MEGA + 1], fp16, tag="megah",
                                      name=f"e1m{h}")
                        st["e1m"][hh] = e1m
                        nc.gpsimd.memset(e1m[:, MEGA:MEGA + 1], 0.0)
                        for qb in range(NT):
                            W = 128 * (qb + 1)
                            off = OFFS[qb]
                            sp = mmps.tile([128, S], fp32, tag="mm",
                                           name=f"spA{h}_{qb}")
                            for j in range(0, W, 512):
                                je = min(j + 512, W)
                                nc.tensor.matmul(
                                    out=sp[:, j:je],
                                    lhsT=qmT[hp][hr, qb * 128:(qb + 1) * 128],
                                    rhs=kmT[hp][hr, j:je],
                                    start=True, stop=True,
                                )
                            nc.scalar.activation(
                                out=e1m[:, off:off + W], in_=sp[:, :W],
                                func=mybir.ActivationFunctionType.Exp,
                                scale=0.125, bias=negg_sb[:, H:H + 1])
                    for hh in range(2):
                        st_e1m = st["e1m"][hh]
                        ranges = [(0, MEGA, nc.vector)]
                        if scan_split:
                            eng = (nc.gpsimd if scan_split_engine == "gpsimd"
                                   else nc.vector)
                            ranges = [(0, scan_split, nc.vector),
                                      (scan_split, MEGA, eng)]
                        for (a, b, eng) in ranges:
                            fwd = st_e1m[:, 0:MEGA]
                            rev = bass.AP(fwd.tensor, fwd.offset + b - 1,
                                          [list(fwd.ap[0]), [-1, b - a]])
                            eng.tensor_tensor_scan(
                                out=rev,
                                data0=rstrev_sb[:, MEGA - b:MEGA - a],
                                data1=rev, initial=0.0,
                                op0=mybir.AluOpType.mult,
                                op1=mybir.AluOpType.add)
                    # Per-head decay scale: srgh = rsqrt(tot) * (-g*cm), one
                    # value per (partition=q, qb). Seeded by the fp32
                    # bit-shift sqrt of rtots, refined with one Newton
                    # iteration (y1 = y0*(1.5 - 0.5*tot*y0^2)).
                    for hh in range(2):
                        h = 2 * hp + hh
                        rtots = sstats.tile([128, 5 * NT], fp32, tag="rtots",
                                            name=f"rtots{h}")
                        st["rtots"][hh] = rtots
                        rt = rtots[:, 0:NT]
                        tot8 = rtots[:, NT:2 * NT]
                        ysq = rtots[:, 2 * NT:3 * NT]
                        ycor = rtots[:, 3 * NT:4 * NT]
                        srgh = rtots[:, 4 * NT:5 * NT]
                        st["srgh"][hh] = srgh
                        for qb in range(NT):
                            off = OFFS[qb]
                            nc.vector.reciprocal(
                                out=rt[:, qb:qb + 1],
                                in_=st["e1m"][hh][:, off:off + 1])
                        nc.vector.reciprocal(out=tot8, in_=rt)
                        y0u = rt.bitcast(mybir.dt.uint32)
                        nc.vector.tensor_scalar(
                            out=y0u, in0=y0u, scalar1=1, scalar2=None,
                            op0=mybir.AluOpType.logical_shift_right,
                            op1=mybir.AluOpType.bypass)
                        nc.vector.tensor_scalar(
                            out=y0u, in0=y0u, scalar1=0x1FBC0000,
                            scalar2=None,
                            op0=mybir.AluOpType.add,
                            op1=mybir.AluOpType.bypass)
                        nc.vector.tensor_tensor(
                            out=ysq, in0=rt, in1=rt, op=mybir.AluOpType.mult)
                        nc.vector.tensor_tensor(
                            out=ysq, in0=ysq, in1=tot8,
                            op=mybir.AluOpType.mult)
                        nc.vector.tensor_scalar(
                            out=ycor, in0=ysq, scalar1=-0.5, scalar2=1.5,
                            op0=mybir.AluOpType.mult,
                            op1=mybir.AluOpType.add)
                        nc.vector.tensor_tensor(
                            out=ycor, in0=ycor, in1=rt,
                            op=mybir.AluOpType.mult)
                        nc.vector.tensor_scalar(
                            out=srgh, in0=ycor,
                            scalar1=negg_sb[:, h:h + 1], scalar2=None,
                            op0=mybir.AluOpType.mult,
                            op1=mybir.AluOpType.bypass)
                    for hh in range(2):
                        h = 2 * hp + hh
                        u = mb.tile([128, MEGA], bf16,
                                    tag="megab", name=f"u{h}")
                        st["u"][hh] = u
                        u0_eng = nc.gpsimd if pool_u0 else nc.vector
                        u0_eng.tensor_tensor(
                            out=u[:], in0=st["e1m"][hh][:, 1:MEGA + 1],
                            in1=posflip_sb[:], op=mybir.AluOpType.mult)
                    for hh in range(2):
                        uv = st["u"][hh][:].bitcast(mybir.dt.uint16)
                        eng = nc.gpsimd if pool_shift else nc.vector
                        eng.tensor_scalar(
                            out=uv, in0=uv, scalar1=1, scalar2=None,
                            op0=mybir.AluOpType.logical_shift_right,
                            op1=mybir.AluOpType.bypass)
                    for hh in range(2):
                        h = 2 * hp + hh
                        eff = mb.tile([128, MEGA], bf16, tag="megab",
                                      name=f"eff{h}")
                        st["eff"][hh] = eff
                        for qb in range(NT):
                            W = 128 * (qb + 1)
                            off = OFFS[qb]
                            nc.scalar.activation(
                                out=eff[:, off:off + W],
                                in_=st["u"][hh][:, off:off + W],
                                func=mybir.ActivationFunctionType.Exp,
                                scale=st["srgh"][hh][:, qb:qb + 1],
                                bias=negg_sb[:, H + 1:H + 2])

                def emit_B(hp):
                    st = state.pop(hp)
                    om_ps = omps.tile([128, S], fp32, tag="om",
                                      name=f"om_ps{hp}")
                    e2m = {}
                    tot2s = {}
                    for hh in range(2):
                        h = 2 * hp + hh
                        hr = slice(hh * 64, (hh + 1) * 64)
                        e2m[hh] = e2p.tile([128, MEGA], bf16, tag="e2mega",
                                           name=f"e2m{h}")
                        tot2s[hh] = sstats.tile([128, NT], fp32, tag="tot2s",
                                                name=f"tot2s{h}")
                        for qb in range(NT):
                            W = 128 * (qb + 1)
                            off = OFFS[qb]
                            sp = mmps.tile([128, S], fp32, tag="mm",
                                           name=f"spB{h}_{qb}")
                            for j in range(0, W, 512):
                                je = min(j + 512, W)
                                nc.tensor.matmul(
                                    out=sp[:, j:je],
                                    lhsT=qmT[hp][hr, qb * 128:(qb + 1) * 128],
                                    rhs=kmT[hp][hr, j:je],
                                    start=True, stop=True,
                                )
                            # diagonal-block causal mask folded into the PE:
                            # I @ negtri accumulates -1e30 above the diagonal
                            # (eff there is exactly 1/8, so it stays -1e29
                            # after the s2 multiply).
                            nc.tensor.matmul(
                                out=sp[:, W - 128:W],
                                lhsT=ident_sb[:],
                                rhs=negtri_sb[:],
                                start=False, stop=True,
                                skip_group_check=True,
                            )
                            # s2 = s * eff in place in PSUM, then exp
                            nc.vector.tensor_tensor(
                                out=sp[:, :W], in0=sp[:, :W],
                                in1=st["eff"][hh][:, off:off + W],
                                op=mybir.AluOpType.mult)
                            nc.scalar.activation(
                                out=e2m[hh][:, off:off + W],
                                in_=sp[:, :W],
                                func=mybir.ActivationFunctionType.Exp,
                                accum_out=tot2s[hh][:, qb:qb + 1])
                    for hh in range(2):
                        h = 2 * hp + hh
                        hr = slice(hh * 64, (hh + 1) * 64)
                        rt2 = sstats.tile([128, NT], fp32, tag="rt2",
                                          name=f"rt2_{h}")
                        nc.vector.reciprocal(out=rt2[:], in_=tot2s[hh][:])
                        norm_eng = nc.gpsimd if pool_norm else nc.vector
                        for qb in range(NT):
                            W = 128 * (qb + 1)
                            off = OFFS[qb]
                            norm_eng.tensor_scalar_mul(
                                out=e2m[hh][:, off:off + W],
                                in0=e2m[hh][:, off:off + W],
                                scalar1=rt2[:, qb:qb + 1])
                        # DMA-transpose e2 into a padded per-k-block layout:
                        # block (c, qb) lands at col c*896 + 128*qb, i.e.
                        # col(c, q) = c*896 + q. P^T rows for k-block c are
                        # then contiguous in q, so PV is 12 big chunked
                        # matmuls per head instead of 36 small ones.
                        e2TK = ep.tile([128, 7 * 896 + S], bf16, tag="e2TK",
                                       name=f"e2TK{h}")
                        for qb in range(NT):
                            W = 128 * (qb + 1)
                            off = OFFS[qb]
                            base = e2TK[:]
                            out_ap = bass.AP(
                                base.tensor, base.offset + 128 * qb,
                                [list(base.ap[0]), [896, qb + 1], [1, 128]])
                            nc.sync.dma_start_transpose(
                                out=out_ap, in_=e2m[hh][:, off:off + W])
                        for (j0, j1) in ((0, 512), (512, S)):
                            cs = [c for c in range(NT) if 128 * c < j1]
                            for c in cs:
                                q0 = max(j0, 128 * c)
                                nc.tensor.matmul(
                                    out=om_ps[hr, q0:j1],
                                    lhsT=vm[c][:, h * 64:(h + 1) * 64],
                                    rhs=e2TK[:, c * 896 + q0:c * 896 + j1],
                                    start=(c == 0), stop=(c == cs[-1]),
                                    skip_group_check=True,
                                )
                    nc.scalar.copy(out=omT[hp][:], in_=om_ps[:])

                # Software pipeline: A(hp+1) is emitted before B(hp) so the
                # PE has independent matmul work while pair hp's scan/decay
                # chain runs on DVE/Pool/ACT.
                emit_A(0)
                if n_hp > 1:
                    emit_A(1)
                proj_v()
                for hp in range(n_hp):
                    emit_B(hp)
                    if hp + 2 < n_hp:
                        emit_A(hp + 2)

            # ================= phase 3: output projection =================
            if not do_out:
                for name in dbg_dump:
                    t = {**{f"qmT{c}": qmT[c] for c in range(8)},
                         **{f"kmT{c}": kmT[c] for c in range(8)},
                         **{f"vm{c}": vm[c] for c in range(8)},
                         **{f"omT{c}": omT[c] for c in range(8)}}[name]
                    dd = nc.dram_tensor(f"dbg_{name}", list(t.shape),
                                        t.dtype, kind="ExternalOutput")
                    nc.sync.dma_start(out=dd.ap()[:, :], in_=t[:])
            else:
                with tc.tile_pool(name="p3w", bufs=1) as wop, \
                     tc.tile_pool(name="p3o", bufs=2) as outp:
                    wo_t = [wop.tile([128, D], bf16, tag=f"wo{c}", name=f"wo{c}")
                            for c in range(8)]
                    for c in range(8):
                        nc.sync.dma_start(out=wo_t[c][:],
                                          in_=woT.ap()[c * 128:(c + 1) * 128, :])
                    for sb in range(8):
                        ps = mmps.tile([128, D], fp32, tag="mm", name=f"ps3{sb}")
                        for j in range(0, D, 512):
                            for c in range(8):
                                nc.tensor.matmul(
                                    out=ps[:, j:j + 512],
                                    lhsT=omT[c][:, sb * 128:(sb + 1) * 128],
                                    rhs=wo_t[c][:, j:j + 512],
                                    start=(c == 0), stop=(c == 7),
                                )
                        st = outp.tile([128, D], fp32, tag="ost", name=f"ost{sb}")
                        nc.scalar.copy(out=st[:], in_=ps[:])
                        nc.sync.dma_start(out=out_d.ap()[sb * 128:(sb + 1) * 128, :],
                                          in_=st[:])
    nc.compile()
    return nc


def _host_constants():
    p = np.arange(128, dtype=np.int64)[:, None]
    posflip = np.zeros((128, MEGA), np.float32)
    rstrev = np.ones((128, MEGA), np.float32)
    for qb in range(NT):
        W = 128 * (qb + 1)
        off = OFFS[qb]
        k = np.arange(W, dtype=np.int64)[None, :]
        # clamp to >=0: above the diagonal (masked region, incl. the one
        # cross-segment suffix read at k=W-1) u becomes exactly +0.
        posflip[:, off:off + W] = np.maximum(
            (128 * qb + p) - k, 0).astype(np.float32)
        # inclusive scan with per-row reset AT the diagonal: the scan value
        # there restarts from e1[diag], so segment-start values are masked
        # row totals and garbage above the diagonal never crosses.
        for pp_ in range(128):
            rstrev[pp_, MEGA - 1 - (off + 128 * qb + pp_)] = 0.0
    jj = np.arange(128)[None, :]
    negtri = np.where(jj > p, -1e30, 0.0).astype(BF16)
    ident = np.eye(128, dtype=np.float32)
    return (posflip.astype(np.float16), rstrev.astype(np.float16), negtri,
            ident.astype(BF16))


def _softplus(x):
    return np.log1p(np.exp(-np.abs(x))) + np.maximum(x, 0.0)


def _make_in_maps(inputs):
    posflip, rstrev, negtri, ident = _host_constants()
    g = _softplus(np.asarray(inputs["gammas"], np.float32).reshape(H))
    cm = 2.0 ** (0x1FBC / 128.0)  # bf16 shift-sqrt correction
    negg = np.zeros((128, H + 2), np.float32)
    negg[:, :H] = -g[None, :] * cm
    negg[:, H] = LN1_16
    negg[:, H + 1] = np.log(1.0 / 8.0)
    ones1 = np.ones((1, 128), BF16)

    in_maps = []
    for t, tag in ((0, "m"), (1, "c")):
        key = "mean" if t == 0 else "cov"
        wq = np.asarray(inputs[f"wq_{tag}"], np.float32)
        wk = np.asarray(inputs[f"wk_{tag}"], np.float32)
        wv = np.asarray(inputs[f"wv_{tag}"], np.float32)
        wo = np.asarray(inputs[f"wo_{tag}"], np.float32)
        bq = np.asarray(inputs[f"bq_{tag}"], np.float32)
        bk = np.asarray(inputs[f"bk_{tag}"], np.float32)
        bv = np.asarray(inputs[f"bv_{tag}"], np.float32)
        bqk = np.concatenate([bq.reshape(8, 128).T, bk.reshape(8, 128).T],
                             axis=1).astype(np.float32)
        wqT = np.ascontiguousarray(wq.T).astype(BF16)
        wkT = np.ascontiguousarray(wk.T).astype(BF16)
        wvT = np.ascontiguousarray(wv.T).astype(BF16)
        woT = np.ascontiguousarray(wo.T).astype(BF16)
        bvrow = bv.reshape(1, D).astype(BF16)
        for b in range(B):
            xq = np.asarray(inputs[f"query_{key}"][b], np.float32)
            xk = np.asarray(inputs[f"key_{key}"][b], np.float32)
            xv = np.asarray(inputs[f"values_{key}"][b], np.float32)
            in_maps.append({
                "xqT": np.ascontiguousarray(xq.T).astype(BF16),
                "xkT": np.ascontiguousarray(xk.T).astype(BF16),
                "xvT": np.ascontiguousarray(xv.T).astype(BF16),
                "wqT": wqT, "wkT": wkT, "wvT": wvT, "woT": woT,
                "bqk": bqk, "bvrow": bvrow, "ones1": ones1,
                "negg": negg, "posflip": posflip, "rstrev": rstrev,
                "negtri": negtri, "ident": ident,
            })
    return in_maps


def _numpy_reference(inputs):
    """Pure-numpy fallback replicating reference.py (used only if the mask is
    not causal-tril, which the staged problem never produces)."""
    def lin(x, w, b):
        return x @ w.T + b

    def split_heads(x):
        return x.reshape(B, S, H, DK).transpose(0, 2, 1, 3)

    def merge_heads(x):
        return x.transpose(0, 2, 1, 3).reshape(B, S, D)

    def softmax(x):
        m = x.max(-1, keepdims=True)
        e = np.exp(x - m)
        return e / e.sum(-1, keepdims=True)

    mask_f = np.asarray(inputs["mask"], np.float32)
    idx = np.arange(S, dtype=np.float32)
    pos = np.abs(idx[None, :] - idx[:, None])
    g = -_softplus(np.asarray(inputs["gammas"], np.float32))[None]

    outs = []
    for tag, key in (("m", "mean"), ("c", "cov")):
        q = split_heads(lin(np.asarray(inputs[f"query_{key}"], np.float32),
                            np.asarray(inputs[f"wq_{tag}"]), np.asarray(inputs[f"bq_{tag}"])))
        k = split_heads(lin(np.asarray(inputs[f"key_{key}"], np.float32),
                            np.asarray(inputs[f"wk_{tag}"]), np.asarray(inputs[f"bk_{tag}"])))
        v = split_heads(lin(np.asarray(inputs[f"values_{key}"], np.float32),
                            np.asarray(inputs[f"wv_{tag}"]), np.asarray(inputs[f"bv_{tag}"])))
        sc = np.einsum('bhqd,bhkd->bhqk', q, k) / np.sqrt(DK)
        pm = softmax(np.where(mask_f == 0, -1e32, sc)) * mask_f
        cum = np.cumsum(pm, -1)
        tot = pm.sum(-1, keepdims=True)
        dist = np.sqrt(np.clip((tot - cum) * pos, 0.0, None))
        effd = np.clip(np.exp(dist * g), 1e-5, 1e5)
        s = np.where(mask_f == 0, -1e32, sc * effd)
        p = softmax(s)
        o = merge_heads(np.einsum('bhqk,bhkd->bhqd', p, v))
        outs.append(lin(o, np.asarray(inputs[f"wo_{tag}"]),
                        np.asarray(inputs[f"bo_{tag}"])))
    return np.stack(outs, 0).astype(np.float32)


def _ensure_ntff_hook():
    """The container's antenv lacks axon_hooks; bass_utils imports it
    unguarded when trace=True. Provide it, backed by the ctypes NTFF
    profiler from trn_agent_boot when available."""
    import sys as _sys
    import types as _types
    if "antenv.axon_hooks" in _sys.modules:
        return
    hook = None
    try:
        from trn_agent_boot.trn_boot import _ntff_profile_via_ctypes
        hook = _ntff_profile_via_ctypes("/opt/axon/libaxon_pjrt.so")
    except Exception:
        hook = None
    mod = _types.ModuleType("antenv.axon_hooks")
    mod.get_axon_ntff_profile_hook = lambda: hook
    mod.set_axon_ntff_profile_hook = lambda h: None
    _sys.modules["antenv.axon_hooks"] = mod
    try:
        import antenv
        antenv.axon_hooks = mod
    except Exception:
        pass


def run(inputs, trace=False):
    if trace:
        _ensure_ntff_hook()
    mask = np.asarray(inputs["mask"]).reshape(S, S)
    if not np.array_equal(mask, np.tril(np.ones((S, S), mask.dtype))):
        return _numpy_reference(inputs), None

    if "nc" not in _cache:
        _cache["nc"] = build_bass()
    nc = _cache["nc"]
    in_maps = _make_in_maps(inputs)
    res = run_bass_kernel_spmd(nc, in_maps, core_ids=list(range(8)), trace=trace)

    bo_m = np.asarray(inputs["bo_m"], np.float32)
    bo_c = np.asarray(inputs["bo_c"], np.float32)
    out = np.zeros((2, B, S, D), np.float32)
    for t in range(2):
        bo = bo_m if t == 0 else bo_c
        for b in range(B):
            out[t, b] = res.results[t * 4 + b]["out"] + bo[None, :]
    return out, res


def kernel(**inputs) -> np.ndarray:
    out, _ = run(inputs, trace=False)
    return out
